# Initial kernel scaffold
#
"""Deep Lagrangian Network kernel for 8 Trainium2 NeuronCores.

Data-parallel over the batch (32768 samples -> 4096/core). Per core:
  - 3-layer ReLU MLP forward (fp32r matmuls, feature-major layout)
  - forward-mode Jacobian: 7 tangent columns propagated through the trunk
    (seed_k = relu'(z1) * W0[:,k], masked by relu' at each layer)
  - heads (diag 7 + tril 21) for forward and all tangents
  - PE transposes to sample-major, then scatter-assembly of the 11 outputs
    (L, L_dq, transposes, H = L L^T, L_dt = L_dq . qd) as contiguous
    per-sample records, DMA'd out as dense blocks.

kernel(**inputs) takes the FULL unsharded inputs and returns the full
11-tuple matching the reference.
"""
import numpy as np

import concourse.bacc as bacc
import concourse.tile as tile
import concourse.mybir as mybir
from concourse.bass_utils import run_bass_kernel_spmd
from concourse import masks

F32 = mybir.dt.float32
F32R = mybir.dt.float32r
AF = mybir.ActivationFunctionType
OP = mybir.AluOpType

N_DOF = 7
N_TRIL = 21
NH = 28          # head rows: 7 diag + 21 tril
HID = 512
B = 32768
NCORES = 8
BC = B // NCORES          # 4096 samples per core
NS = 512                  # samples per chunk
NCH = BC // NS            # 8 chunks
NMT = HID // 128          # 4 partition tiles per 512 features
# tril row offsets: row r (1..6) covers tril indices off_r .. off_r+r-1
TRIL_OFF = [r * (r - 1) // 2 for r in range(8)]

_NC_CACHE = {}


def _build_nc():
    nc = bacc.Bacc(None, target_bir_lowering=False)

    # ---- per-core DRAM inputs (host pre-transposed layouts)
    q_d = nc.dram_tensor("q", [BC, N_DOF], F32, kind="ExternalInput")
    qd_d = nc.dram_tensor("qd", [BC, N_DOF], F32, kind="ExternalInput")
    w0t_d = nc.dram_tensor("w0t", [N_DOF, HID], F32, kind="ExternalInput")
    w0c_d = nc.dram_tensor("w0c", [HID, N_DOF], F32, kind="ExternalInput")
    w1t_d = nc.dram_tensor("w1t", [HID, HID], F32, kind="ExternalInput")
    w2t_d = nc.dram_tensor("w2t", [HID, HID], F32, kind="ExternalInput")
    wht_d = nc.dram_tensor("wht", [HID, NH], F32, kind="ExternalInput")
    b0_d = nc.dram_tensor("b0m", [128, NMT], F32, kind="ExternalInput")
    b1_d = nc.dram_tensor("b1m", [128, NMT], F32, kind="ExternalInput")
    b2_d = nc.dram_tensor("b2m", [128, NMT], F32, kind="ExternalInput")
    bh_d = nc.dram_tensor("bh", [NH, 1], F32, kind="ExternalInput")

    # ---- per-core DRAM outputs
    o_diag = nc.dram_tensor("o_diag", [BC, 7], F32, kind="ExternalOutput")
    o_tril = nc.dram_tensor("o_tril", [BC, 21], F32, kind="ExternalOutput")
    o_ddq = nc.dram_tensor("o_ddq", [BC, 49], F32, kind="ExternalOutput")
    o_tdq = nc.dram_tensor("o_tdq", [BC, 147], F32, kind="ExternalOutput")
    o_L = nc.dram_tensor("o_L", [BC, 49], F32, kind="ExternalOutput")
    o_Lt = nc.dram_tensor("o_Lt", [BC, 49], F32, kind="ExternalOutput")
    o_ldq = nc.dram_tensor("o_ldq", [BC, 343], F32, kind="ExternalOutput")
    o_ldqt = nc.dram_tensor("o_ldqt", [BC, 343], F32, kind="ExternalOutput")
    o_H = nc.dram_tensor("o_H", [BC, 49], F32, kind="ExternalOutput")
    o_ldt = nc.dram_tensor("o_ldt", [BC, 49], F32, kind="ExternalOutput")
    o_ldtt = nc.dram_tensor("o_ldtt", [BC, 49], F32, kind="ExternalOutput")

    with tile.TileContext(nc) as tc:
        with (
            tc.tile_pool(name="wp", bufs=1) as wp,
            tc.tile_pool(name="act", bufs=2) as act,
            tc.tile_pool(name="msk", bufs=2) as msk,
            tc.tile_pool(name="tan", bufs=2) as tan,
            tc.tile_pool(name="hd", bufs=2) as hd,
            tc.tile_pool(name="stg", bufs=3) as stg,
            tc.tile_pool(name="pmm", bufs=4, space="PSUM") as pmm,
            tc.tile_pool(name="php", bufs=2, space="PSUM") as php,
            tc.tile_pool(name="ptr", bufs=2, space="PSUM") as ptr,
        ):
            # ================= constants / weights =================
            w1f = [wp.tile([128, HID], F32, tag=f"w1f{k}", name=f"w1f{k}")
                   for k in range(NMT)]
            w2f = [wp.tile([128, HID], F32, tag=f"w2f{k}", name=f"w2f{k}")
                   for k in range(NMT)]
            w1r = [wp.tile([128, HID], F32R, tag=f"w1r{k}", name=f"w1r{k}")
                   for k in range(NMT)]
            w2r = [wp.tile([128, HID], F32R, tag=f"w2r{k}", name=f"w2r{k}")
                   for k in range(NMT)]
            whf = [wp.tile([128, NH], F32, tag=f"whf{k}", name=f"whf{k}")
                   for k in range(NMT)]
            whr = [wp.tile([128, NH], F32R, tag=f"whr{k}", name=f"whr{k}")
                   for k in range(NMT)]
            w0c = [wp.tile([128, N_DOF], F32, tag=f"w0c{k}", name=f"w0c{k}")
                   for k in range(NMT)]
            for k in range(NMT):
                sl = slice(k * 128, (k + 1) * 128)
                nc.sync.dma_start(w1f[k][:], w1t_d[sl, :])
                nc.sync.dma_start(w2f[k][:], w2t_d[sl, :])
                nc.sync.dma_start(whf[k][:], wht_d[sl, :])
                nc.sync.dma_start(w0c[k][:], w0c_d[sl, :])
                nc.vector.tensor_copy(w1r[k][:], w1f[k][:])
                nc.vector.tensor_copy(w2r[k][:], w2f[k][:])
                nc.vector.tensor_copy(whr[k][:], whf[k][:])
            w0f = wp.tile([N_DOF, HID], F32, tag="w0f")
            nc.sync.dma_start(w0f[:], w0t_d[:])
            w0r = wp.tile([N_DOF, HID], F32R, tag="w0r")
            nc.vector.tensor_copy(w0r[:], w0f[:])
            b0t = wp.tile([128, NMT], F32, tag="b0t")
            nc.sync.dma_start(b0t[:], b0_d[:])
            b1t = wp.tile([128, NMT], F32, tag="b1t")
            nc.sync.dma_start(b1t[:], b1_d[:])
            b2t = wp.tile([128, NMT], F32, tag="b2t")
            nc.sync.dma_start(b2t[:], b2_d[:])
            bht = wp.tile([NH, 1], F32, tag="bht")
            nc.sync.dma_start(bht[:], bh_d[:])
            ident = wp.tile([NH, NH], F32, tag="ident")
            masks.make_identity(nc, ident[:])

            # ================= main loop over chunks =================
            for c in range(NCH):
                s0 = c * NS
                # --- load q chunk transposed [7, NS], round to f32r
                qt = act.tile([N_DOF, NS], F32, tag="qt", name="qt")
                nc.sync.dma_start(qt[:], q_d[s0:s0 + NS, :].rearrange("s d -> d s"))
                qtr = act.tile([N_DOF, NS], F32R, tag="qtr", name="qtr")
                nc.vector.tensor_copy(qtr[:], qt[:])

                # --- layer 1 forward: h1 = relu(W0 @ qT + b0)
                h1 = []
                for mt in range(NMT):
                    zp = pmm.tile([128, NS], F32, tag="mm", name="zp")
                    nc.tensor.matmul(zp[:], w0r[:, mt * 128:(mt + 1) * 128],
                                     qtr[:], start=True, stop=True)
                    h = act.tile([128, NS], F32R, tag=f"h1_{mt}", name=f"h1_{mt}")
                    nc.scalar.activation(h[:], zp[:], AF.Relu,
                                         bias=b0t[:, mt:mt + 1], scale=1.0)
                    h1.append(h)

                # --- layer 2 forward
                h2, d2 = [], []
                for mt in range(NMT):
                    zp = pmm.tile([128, NS], F32, tag="mm", name="zp")
                    for kc in range(NMT):
                        nc.tensor.matmul(zp[:],
                                         w1r[kc][:, mt * 128:(mt + 1) * 128],
                                         h1[kc][:],
                                         start=(kc == 0), stop=(kc == NMT - 1))
                    h = act.tile([128, NS], F32R, tag=f"h2_{mt}", name=f"h2_{mt}")
                    nc.scalar.activation(h[:], zp[:], AF.Relu,
                                         bias=b1t[:, mt:mt + 1], scale=1.0)
                    h2.append(h)
                    d = msk.tile([128, NS], F32, tag=f"d2_{mt}", name=f"d2_{mt}")
                    nc.gpsimd.tensor_single_scalar(d[:], h[:].bitcast(F32), 0.0,
                                                   OP.is_gt)
                    d2.append(d)

                # --- layer 3 forward
                h3, d3 = [], []
                for mt in range(NMT):
                    zp = pmm.tile([128, NS], F32, tag="mm", name="zp")
                    for kc in range(NMT):
                        nc.tensor.matmul(zp[:],
                                         w2r[kc][:, mt * 128:(mt + 1) * 128],
                                         h2[kc][:],
                                         start=(kc == 0), stop=(kc == NMT - 1))
                    h = act.tile([128, NS], F32R, tag=f"h3_{mt}", name=f"h3_{mt}")
                    nc.scalar.activation(h[:], zp[:], AF.Relu,
                                         bias=b2t[:, mt:mt + 1], scale=1.0)
                    h3.append(h)
                    d = msk.tile([128, NS], F32, tag=f"d3_{mt}", name=f"d3_{mt}")
                    nc.gpsimd.tensor_single_scalar(d[:], h[:].bitcast(F32), 0.0,
                                                   OP.is_gt)
                    d3.append(d)

                # --- forward head: [diag relu'd; tril] in fm layout [28, NS]
                hp = php.tile([NH, NS], F32, tag="hp", name="hp")
                for kc in range(NMT):
                    nc.tensor.matmul(hp[:], whr[kc][:], h3[kc][:],
                                     start=(kc == 0), stop=(kc == NMT - 1))
                hf = hd.tile([NH, NS], F32, tag="hf", name="hf")
                nc.scalar.activation(hf[:], hp[:], AF.Identity,
                                     bias=bht[:], scale=1.0)
                nc.scalar.activation(hf[0:7, :], hp[0:7, :], AF.Relu,
                                     bias=bht[0:7, :], scale=1.0)

                # --- tangent propagation, one tangent direction k at a time
                hts = []
                for k in range(N_DOF):
                    seeds = []
                    for mt in range(NMT):
                        sd = tan.tile([128, NS], F32R, tag=f"sd_{mt}",
                                      name=f"sd_{mt}")
                        nc.gpsimd.tensor_scalar(sd[:], h1[mt][:].bitcast(F32),
                                                0.0, w0c[mt][:, k:k + 1],
                                                OP.is_gt, OP.mult)
                        seeds.append(sd)
                    a2 = []
                    for mt in range(NMT):
                        zp = pmm.tile([128, NS], F32, tag="mm", name="zp")
                        for kc in range(NMT):
                            nc.tensor.matmul(zp[:],
                                             w1r[kc][:, mt * 128:(mt + 1) * 128],
                                             seeds[kc][:],
                                             start=(kc == 0), stop=(kc == NMT - 1))
                        a = tan.tile([128, NS], F32R, tag=f"a2_{mt}",
                                     name=f"a2_{mt}")
                        nc.vector.tensor_tensor(a[:], zp[:], d2[mt][:], OP.mult)
                        a2.append(a)
                    a3 = []
                    for mt in range(NMT):
                        zp = pmm.tile([128, NS], F32, tag="mm", name="zp")
                        for kc in range(NMT):
                            nc.tensor.matmul(zp[:],
                                             w2r[kc][:, mt * 128:(mt + 1) * 128],
                                             a2[kc][:],
                                             start=(kc == 0), stop=(kc == NMT - 1))
                        a = tan.tile([128, NS], F32R, tag=f"a3_{mt}",
                                     name=f"a3_{mt}")
                        nc.vector.tensor_tensor(a[:], zp[:], d3[mt][:], OP.mult)
                        a3.append(a)
                    hp_k = php.tile([NH, NS], F32, tag="hp", name="hp_k")
                    for kc in range(NMT):
                        nc.tensor.matmul(hp_k[:], whr[kc][:], a3[kc][:],
                                         start=(kc == 0), stop=(kc == NMT - 1))
                    ht = hd.tile([NH, NS], F32, tag=f"ht_{k}", name=f"ht_{k}")
                    nc.scalar.copy(ht[:], hp_k[:])
                    hts.append(ht)

                # --- per 128-sample tile: transpose + assemble + DMA out
                for p in range(NS // 128):
                    sp = s0 + p * 128
                    csl = slice(p * 128, (p + 1) * 128)
                    tp = ptr.tile([128, 8 * NH], F32, tag="tp", name="tp")
                    nc.tensor.transpose(tp[:, 0:NH], hf[:, csl], ident[:])
                    for k in range(N_DOF):
                        nc.tensor.transpose(tp[:, (k + 1) * NH:(k + 2) * NH],
                                            hts[k][:, csl], ident[:])
                    tp3 = tp[:].rearrange("p (m i) -> p m i", m=8, i=NH)

                    # Dd mask for tangent diag rows (sample-major)
                    ddt = stg.tile([128, 7], F32, tag="ddt", name="ddt")
                    nc.vector.tensor_single_scalar(ddt[:], tp[:, 0:7], 0.0,
                                                   OP.is_gt)
                    nc.vector.tensor_tensor(
                        tp3[:, 1:8, 0:7], tp3[:, 1:8, 0:7],
                        ddt[:].unsqueeze(1).broadcast_to((128, 7, 7)), OP.mult)

                    # out_L_diag / out_L_tril
                    s_diag = stg.tile([128, 7], F32, tag="s_diag", name="s_diag")
                    nc.scalar.copy(s_diag[:], tp[:, 0:7])
                    nc.sync.dma_start(o_diag[sp:sp + 128, :], s_diag[:])
                    s_tril = stg.tile([128, 21], F32, tag="s_tril", name="s_tril")
                    nc.scalar.copy(s_tril[:], tp[:, 7:28])
                    nc.sync.dma_start(o_tril[sp:sp + 128, :], s_tril[:])

                    # out_L_diag_dq [i*7+k] and out_L_tril_dq [j*7+k]
                    s_ddq = stg.tile([128, 49], F32, tag="s_ddq", name="s_ddq")
                    nc.vector.tensor_copy(
                        s_ddq[:].rearrange("p (i k) -> p k i", i=7, k=7),
                        tp3[:, 1:8, 0:7])
                    nc.sync.dma_start(o_ddq[sp:sp + 128, :], s_ddq[:])
                    s_tdq = stg.tile([128, 147], F32, tag="s_tdq", name="s_tdq")
                    nc.vector.tensor_copy(
                        s_tdq[:].rearrange("p (j k) -> p k j", j=21, k=7),
                        tp3[:, 1:8, 7:28])
                    nc.sync.dma_start(o_tdq[sp:sp + 128, :], s_tdq[:])

                    # L record [i*7+j]: diag at 8i (+0.1), tril rows
                    s_L = stg.tile([128, 56], F32, tag="s_L", name="s_L")
                    nc.gpsimd.memset(s_L[:], 0.0)
                    nc.vector.tensor_scalar_add(
                        s_L[:].rearrange("p (i x) -> p i x", i=7, x=8)[:, :, 0:1],
                        tp[:, 0:7].unsqueeze(2), 0.1)
                    for r in range(1, 7):
                        nc.vector.tensor_copy(
                            s_L[:, 7 * r:7 * r + r],
                            tp[:, 7 + TRIL_OFF[r]:7 + TRIL_OFF[r] + r])
                    nc.sync.dma_start(o_L[sp:sp + 128, :], s_L[:, 0:49])

                    # L_transp: free-dim transpose of the 7x7 record
                    s_Lt = stg.tile([128, 49], F32, tag="s_Lt", name="s_Lt")
                    nc.gpsimd.tensor_copy(
                        s_Lt[:].rearrange("p (i j) -> p i j", i=7, j=7),
                        s_L[:, 0:49].rearrange("p (i j) -> p j i", i=7, j=7))
                    nc.sync.dma_start(o_Lt[sp:sp + 128, :], s_Lt[:])

                    # L_dq record [i*49 + j*7 + k]
                    s_ldq = stg.tile([128, 392], F32, tag="s_ldq", name="s_ldq")
                    nc.gpsimd.memset(s_ldq[:], 0.0)
                    nc.gpsimd.tensor_copy(
                        s_ldq[:].rearrange("p (i x) -> p i x", i=7, x=56)[:, :, 0:7],
                        s_ddq[:].rearrange("p (i k) -> p i k", i=7, k=7))
                    for r in range(1, 7):
                        nc.gpsimd.tensor_copy(
                            s_ldq[:, 49 * r:49 * r + 7 * r],
                            s_tdq[:, 7 * TRIL_OFF[r]:7 * TRIL_OFF[r] + 7 * r])
                    nc.sync.dma_start(o_ldq[sp:sp + 128, :], s_ldq[:, 0:343])

                    # L_dq_transpose: swap i,j
                    s_ldqt = stg.tile([128, 343], F32, tag="s_ldqt", name="s_ldqt")
                    nc.gpsimd.tensor_copy(
                        s_ldqt[:].rearrange("p (j i k) -> p j i k", j=7, i=7, k=7),
                        s_ldq[:, 0:343].rearrange("p (i j k) -> p j i k",
                                                  i=7, j=7, k=7))
                    nc.sync.dma_start(o_ldqt[sp:sp + 128, :], s_ldqt[:])

                    # H = L @ L^T per sample
                    l3 = s_L[:, 0:49].rearrange("p (i k) -> p i k", i=7, k=7)
                    s_h3 = stg.tile([128, 343], F32, tag="s_h3", name="s_h3")
                    nc.vector.tensor_tensor(
                        s_h3[:].rearrange("p (i j k) -> p i j k", i=7, j=7, k=7),
                        l3.unsqueeze(2).broadcast_to((128, 7, 7, 7)),
                        l3.unsqueeze(1).broadcast_to((128, 7, 7, 7)), OP.mult)
                    s_H = stg.tile([128, 49], F32, tag="s_H", name="s_H")
                    nc.vector.tensor_reduce(
                        s_H[:].rearrange("p (i j) -> p i j", i=7, j=7),
                        s_h3[:].rearrange("p (i j k) -> p i j k", i=7, j=7, k=7),
                        mybir.AxisListType.X, OP.add)
                    nc.sync.dma_start(o_H[sp:sp + 128, :], s_H[:])

                    # L_dt = sum_k L_dq[..., k] * qd[k]
                    qdt = stg.tile([128, 7], F32, tag="qdt", name="qdt")
                    nc.sync.dma_start(qdt[:], qd_d[sp:sp + 128, :])
                    s_l3 = stg.tile([128, 343], F32, tag="s_l3", name="s_l3")
                    nc.vector.tensor_tensor(
                        s_l3[:].rearrange("p (a k) -> p a k", a=49, k=7),
                        s_ldq[:, 0:343].rearrange("p (a k) -> p a k", a=49, k=7),
                        qdt[:].unsqueeze(1).broadcast_to((128, 49, 7)), OP.mult)
                    s_ldt = stg.tile([128, 49], F32, tag="s_ldt", name="s_ldt")
                    nc.vector.tensor_reduce(
                        s_ldt[:],
                        s_l3[:].rearrange("p (a k) -> p a k", a=49, k=7),
                        mybir.AxisListType.X, OP.add)
                    nc.sync.dma_start(o_ldt[sp:sp + 128, :], s_ldt[:])

                    # L_dt_transpose
                    s_ldtt = stg.tile([128, 49], F32, tag="s_ldtt", name="s_ldtt")
                    nc.gpsimd.tensor_copy(
                        s_ldtt[:].rearrange("p (i j) -> p i j", i=7, j=7),
                        s_ldt[:].rearrange("p (i j) -> p j i", i=7, j=7))
                    nc.sync.dma_start(o_ldtt[sp:sp + 128, :], s_ldtt[:])

    nc.compile()
    return nc


def _get_nc():
    if "nc" not in _NC_CACHE:
        _NC_CACHE["nc"] = _build_nc()
    return _NC_CACHE["nc"]


def _prep_in_maps(q, qd, W0, b0, W1, b1, W2, b2, Wd, bd, Wt, bt):
    q = np.ascontiguousarray(np.asarray(q, np.float32)).reshape(B, N_DOF)
    qd = np.ascontiguousarray(np.asarray(qd, np.float32)).reshape(B, N_DOF)
    W0 = np.asarray(W0, np.float32)
    W1 = np.asarray(W1, np.float32)
    W2 = np.asarray(W2, np.float32)
    Wh = np.concatenate([np.asarray(Wd, np.float32),
                         np.asarray(Wt, np.float32)], axis=0)
    shared = {
        "w0t": np.ascontiguousarray(W0.T),
        "w0c": np.ascontiguousarray(W0),
        "w1t": np.ascontiguousarray(W1.T),
        "w2t": np.ascontiguousarray(W2.T),
        "wht": np.ascontiguousarray(Wh.T),
        "b0m": np.ascontiguousarray(np.asarray(b0, np.float32).reshape(NMT, 128).T),
        "b1m": np.ascontiguousarray(np.asarray(b1, np.float32).reshape(NMT, 128).T),
        "b2m": np.ascontiguousarray(np.asarray(b2, np.float32).reshape(NMT, 128).T),
        "bh": np.ascontiguousarray(
            np.concatenate([np.asarray(bd, np.float32),
                            np.asarray(bt, np.float32)]).reshape(NH, 1)),
    }
    in_maps = []
    for c in range(NCORES):
        sl = slice(c * BC, (c + 1) * BC)
        in_maps.append({"q": q[sl], "qd": qd[sl], **shared})
    return in_maps


def run_cores(in_maps):
    nc = _get_nc()
    return run_bass_kernel_spmd(nc, in_maps, core_ids=list(range(NCORES)))


def kernel(q, qd, qdd, W0, b0, W1, b1, W2, b2, Wd, bd, Wt, bt):
    in_maps = _prep_in_maps(q, qd, W0, b0, W1, b1, W2, b2, Wd, bd, Wt, bt)
    res = run_cores(in_maps).results

    def cat(name):
        return np.concatenate([res[c][name] for c in range(NCORES)], axis=0)

    out_L_diag = cat("o_diag")                        # (B, 7)
    out_L_tril = cat("o_tril")                        # (B, 21)
    out_L_diag_dq = cat("o_ddq").reshape(B, 7, 7)
    out_L_tril_dq = cat("o_tdq").reshape(B, 21, 7)
    L = cat("o_L").reshape(B, 7, 7)
    L_dq = cat("o_ldq").reshape(B, 7, 7, 7)
    L_transp = cat("o_Lt").reshape(B, 7, 7)
    L_dq_transpose = cat("o_ldqt").reshape(B, 7, 7, 7)
    H = cat("o_H").reshape(B, 7, 7)
    L_dt = cat("o_ldt").reshape(B, 7, 7)
    L_dt_transpose = cat("o_ldtt").reshape(B, 7, 7)
    return (out_L_diag, out_L_tril, out_L_diag_dq, out_L_tril_dq,
            L, L_dq, L_transp, L_dq_transpose, H, L_dt, L_dt_transpose)


# revision 5
# speedup vs baseline: 64.2950x; 64.2950x over previous
"""Deep Lagrangian Network kernel for 8 Trainium2 NeuronCores.

Data-parallel over the batch (32768 samples -> 4096/core). Per core:
  - 3-layer ReLU MLP forward (fp32r matmuls, feature-major layout)
  - forward-mode Jacobian: 7 tangent columns propagated through the trunk
    (seed_k = relu'(z1) * W0[:,k], masked by relu' at each layer)
  - heads (diag 7 + tril 21) for forward and all tangents
  - PE transposes to sample-major, then scatter-assembly of the 11 outputs
    (L, L_dq, transposes, H = L L^T, L_dt = L_dq . qd) as contiguous
    per-sample records, DMA'd out as dense blocks.

kernel(**inputs) takes the FULL unsharded inputs and returns the full
11-tuple matching the reference.
"""
import numpy as np

import concourse.bacc as bacc
import concourse.tile as tile
import concourse.mybir as mybir
from concourse.bass_utils import run_bass_kernel_spmd
from concourse import masks

F32 = mybir.dt.float32
F32R = mybir.dt.float32r
AF = mybir.ActivationFunctionType
OP = mybir.AluOpType

N_DOF = 7
N_TRIL = 21
NH = 28          # head rows: 7 diag + 21 tril
HID = 512
B = 32768
NCORES = 8
BC = B // NCORES          # 4096 samples per core
NS = 512                  # samples per chunk
NCH = BC // NS            # 8 chunks
NMT = HID // 128          # 4 partition tiles per 512 features
# tril row offsets: row r (1..6) covers tril indices off_r .. off_r+r-1
TRIL_OFF = [r * (r - 1) // 2 for r in range(8)]

_NC_CACHE = {}


def _build_nc(repeat=1):
    nc = bacc.Bacc(None, target_bir_lowering=False)

    # ---- per-core DRAM inputs (host pre-transposed layouts)
    q_d = nc.dram_tensor("q", [BC, N_DOF], F32, kind="ExternalInput")
    qd_d = nc.dram_tensor("qd", [BC, N_DOF], F32, kind="ExternalInput")
    w0t_d = nc.dram_tensor("w0t", [N_DOF, HID], F32, kind="ExternalInput")
    w0c_d = nc.dram_tensor("w0c", [HID, N_DOF], F32, kind="ExternalInput")
    w1t_d = nc.dram_tensor("w1t", [HID, HID], F32, kind="ExternalInput")
    w2t_d = nc.dram_tensor("w2t", [HID, HID], F32, kind="ExternalInput")
    wht_d = nc.dram_tensor("wht", [HID, NH], F32, kind="ExternalInput")
    b0_d = nc.dram_tensor("b0m", [128, NMT], F32, kind="ExternalInput")
    b1_d = nc.dram_tensor("b1m", [128, NMT], F32, kind="ExternalInput")
    b2_d = nc.dram_tensor("b2m", [128, NMT], F32, kind="ExternalInput")
    bh_d = nc.dram_tensor("bh", [NH, 1], F32, kind="ExternalInput")

    # ---- per-core DRAM outputs
    o_diag = nc.dram_tensor("o_diag", [BC, 7], F32, kind="ExternalOutput")
    o_tril = nc.dram_tensor("o_tril", [BC, 21], F32, kind="ExternalOutput")
    o_ddq = nc.dram_tensor("o_ddq", [BC, 49], F32, kind="ExternalOutput")
    o_tdq = nc.dram_tensor("o_tdq", [BC, 147], F32, kind="ExternalOutput")
    o_L = nc.dram_tensor("o_L", [BC, 49], F32, kind="ExternalOutput")
    o_Lt = nc.dram_tensor("o_Lt", [BC, 49], F32, kind="ExternalOutput")
    o_ldq = nc.dram_tensor("o_ldq", [BC, 343], F32, kind="ExternalOutput")
    o_ldqt = nc.dram_tensor("o_ldqt", [BC, 343], F32, kind="ExternalOutput")
    o_H = nc.dram_tensor("o_H", [BC, 49], F32, kind="ExternalOutput")
    o_ldt = nc.dram_tensor("o_ldt", [BC, 49], F32, kind="ExternalOutput")
    o_ldtt = nc.dram_tensor("o_ldtt", [BC, 49], F32, kind="ExternalOutput")

    with tile.TileContext(nc) as tc:
        with (
            tc.tile_pool(name="wp", bufs=1) as wp,
            tc.tile_pool(name="act", bufs=2) as act,
            tc.tile_pool(name="msk", bufs=1) as msk,
            tc.tile_pool(name="tan", bufs=2) as tan,
            tc.tile_pool(name="hd", bufs=2) as hd,
            tc.tile_pool(name="stg", bufs=2) as stg,
            tc.tile_pool(name="pmm", bufs=4, space="PSUM") as pmm,
            tc.tile_pool(name="php", bufs=2, space="PSUM") as php,
            tc.tile_pool(name="ptr", bufs=2, space="PSUM") as ptr,
        ):
            # ================= constants / weights =================
            w1f = [wp.tile([128, HID], F32, tag=f"w1f{k}", name=f"w1f{k}")
                   for k in range(NMT)]
            w2f = [wp.tile([128, HID], F32, tag=f"w2f{k}", name=f"w2f{k}")
                   for k in range(NMT)]
            whf = [wp.tile([128, NH], F32, tag=f"whf{k}", name=f"whf{k}")
                   for k in range(NMT)]
            w1r = [wp.tile([128, HID], F32R, tag=f"w1r{k}", name=f"w1r{k}")
                   for k in range(NMT)]
            w2r = [wp.tile([128, HID], F32R, tag=f"w2r{k}", name=f"w2r{k}")
                   for k in range(NMT)]
            whr = [wp.tile([128, NH], F32R, tag=f"whr{k}", name=f"whr{k}")
                   for k in range(NMT)]
            w0c = [wp.tile([128, N_DOF], F32, tag=f"w0c{k}", name=f"w0c{k}")
                   for k in range(NMT)]
            for k in range(NMT):
                sl = slice(k * 128, (k + 1) * 128)
                nc.sync.dma_start(w1f[k][:], w1t_d[sl, :])
                nc.vector.tensor_copy(w1r[k][:], w1f[k][:])
                nc.sync.dma_start(w2f[k][:], w2t_d[sl, :])
                nc.vector.tensor_copy(w2r[k][:], w2f[k][:])
                nc.sync.dma_start(whf[k][:], wht_d[sl, :])
                nc.vector.tensor_copy(whr[k][:], whf[k][:])
                nc.sync.dma_start(w0c[k][:], w0c_d[sl, :])
            w0f = wp.tile([N_DOF, HID], F32, tag="w0f")
            nc.sync.dma_start(w0f[:], w0t_d[:])
            b0t = wp.tile([128, NMT], F32, tag="b0t")
            nc.sync.dma_start(b0t[:], b0_d[:])
            b1t = wp.tile([128, NMT], F32, tag="b1t")
            nc.sync.dma_start(b1t[:], b1_d[:])
            b2t = wp.tile([128, NMT], F32, tag="b2t")
            nc.sync.dma_start(b2t[:], b2_d[:])
            bht = wp.tile([NH, 1], F32, tag="bht")
            nc.sync.dma_start(bht[:], bh_d[:])
            ident = wp.tile([NH, NH], F32, tag="ident")
            masks.make_identity(nc, ident[:])

            # ================= main loop over chunks =================
            for c in [cc % NCH for cc in range(NCH * repeat)]:
                s0 = c * NS
                # --- load q chunk transposed [7, NS], round to f32r
                qt = act.tile([N_DOF, NS], F32, tag="qt", name="qt")
                nc.sync.dma_start(qt[:], q_d[s0:s0 + NS, :].rearrange("s d -> d s"))

                # --- layer 1 forward: h1 = relu(W0 @ qT + b0)
                h1 = []
                for mt in range(NMT):
                    zp = pmm.tile([128, NS], F32, tag="mm", name="zp")
                    nc.tensor.matmul(zp[:], w0f[:, mt * 128:(mt + 1) * 128],
                                     qt[:], start=True, stop=True)
                    h = act.tile([128, NS], F32, tag=f"h1_{mt}", name=f"h1_{mt}")
                    nc.scalar.activation(h[:], zp[:], AF.Relu,
                                         bias=b0t[:, mt:mt + 1], scale=1.0)
                    h1.append(h)

                # --- layer 2 forward
                h2, d2 = [], []
                for mt in range(NMT):
                    zp = pmm.tile([128, NS], F32, tag="mm", name="zp")
                    for kc in range(NMT):
                        nc.tensor.matmul(zp[:],
                                         w1f[kc][:, mt * 128:(mt + 1) * 128],
                                         h1[kc][:],
                                         start=(kc == 0), stop=(kc == NMT - 1))
                    h = act.tile([128, NS], F32, tag=f"h2_{mt}", name=f"h2_{mt}")
                    nc.scalar.activation(h[:], zp[:], AF.Relu,
                                         bias=b1t[:, mt:mt + 1], scale=1.0)
                    h2.append(h)
                    d = msk.tile([128, NS], F32, tag=f"d2_{mt}", name=f"d2_{mt}")
                    nc.gpsimd.tensor_single_scalar(d[:], h[:], 0.0,
                                                   OP.is_gt)
                    d2.append(d)

                # --- layer 3 forward
                h3, d3 = [], []
                for mt in range(NMT):
                    zp = pmm.tile([128, NS], F32, tag="mm", name="zp")
                    for kc in range(NMT):
                        nc.tensor.matmul(zp[:],
                                         w2f[kc][:, mt * 128:(mt + 1) * 128],
                                         h2[kc][:],
                                         start=(kc == 0), stop=(kc == NMT - 1))
                    h = act.tile([128, NS], F32, tag=f"h3_{mt}", name=f"h3_{mt}")
                    nc.scalar.activation(h[:], zp[:], AF.Relu,
                                         bias=b2t[:, mt:mt + 1], scale=1.0)
                    h3.append(h)
                    d = msk.tile([128, NS], F32, tag=f"d3_{mt}", name=f"d3_{mt}")
                    nc.gpsimd.tensor_single_scalar(d[:], h[:], 0.0,
                                                   OP.is_gt)
                    d3.append(d)

                # --- forward head: [diag relu'd; tril] in fm layout [28, NS]
                hp = php.tile([NH, NS], F32, tag="hp", name="hp")
                for kc in range(NMT):
                    nc.tensor.matmul(hp[:], whf[kc][:], h3[kc][:],
                                     start=(kc == 0), stop=(kc == NMT - 1))
                hf = hd.tile([NH, NS], F32, tag="hf", name="hf")
                nc.scalar.activation(hf[:], hp[:], AF.Identity,
                                     bias=bht[:], scale=1.0)
                nc.scalar.activation(hf[0:7, :], hp[0:7, :], AF.Relu,
                                     bias=bht[0:7, :], scale=1.0)

                # --- tangent propagation, one tangent direction k at a time
                hts = []
                for k in range(N_DOF):
                    seeds = []
                    for mt in range(NMT):
                        sd = tan.tile([128, NS], F32R, tag=f"sd_{mt}",
                                      name=f"sd_{mt}")
                        nc.gpsimd.tensor_scalar(sd[:], h1[mt][:],
                                                0.0, w0c[mt][:, k:k + 1],
                                                OP.is_gt, OP.mult)
                        seeds.append(sd)
                    a2 = []
                    for mt in range(NMT):
                        zp = pmm.tile([128, NS], F32, tag="mm", name="zp")
                        for kc in range(NMT):
                            nc.tensor.matmul(zp[:],
                                             w1r[kc][:, mt * 128:(mt + 1) * 128],
                                             seeds[kc][:],
                                             start=(kc == 0), stop=(kc == NMT - 1))
                        a = tan.tile([128, NS], F32R, tag=f"a2_{mt}",
                                     name=f"a2_{mt}")
                        nc.vector.tensor_tensor(a[:], zp[:], d2[mt][:], OP.mult)
                        a2.append(a)
                    a3 = []
                    for mt in range(NMT):
                        zp = pmm.tile([128, NS], F32, tag="mm", name="zp")
                        for kc in range(NMT):
                            nc.tensor.matmul(zp[:],
                                             w2r[kc][:, mt * 128:(mt + 1) * 128],
                                             a2[kc][:],
                                             start=(kc == 0), stop=(kc == NMT - 1))
                        a = tan.tile([128, NS], F32R, tag=f"a3_{mt}",
                                     name=f"a3_{mt}")
                        nc.vector.tensor_tensor(a[:], zp[:], d3[mt][:], OP.mult)
                        a3.append(a)
                    hp_k = php.tile([NH, NS], F32, tag="hp", name="hp_k")
                    for kc in range(NMT):
                        nc.tensor.matmul(hp_k[:], whr[kc][:], a3[kc][:],
                                         start=(kc == 0), stop=(kc == NMT - 1))
                    ht = hd.tile([NH, NS], F32, tag=f"ht_{k}", name=f"ht_{k}")
                    nc.scalar.copy(ht[:], hp_k[:])
                    hts.append(ht)

                # --- per 128-sample tile: transpose + assemble + DMA out
                for p in range(NS // 128):
                    sp = s0 + p * 128
                    csl = slice(p * 128, (p + 1) * 128)
                    tp = ptr.tile([128, 8 * NH], F32, tag="tp", name="tp")
                    nc.tensor.transpose(tp[:, 0:NH], hf[:, csl], ident[:])
                    for k in range(N_DOF):
                        nc.tensor.transpose(tp[:, (k + 1) * NH:(k + 2) * NH],
                                            hts[k][:, csl], ident[:])
                    tp3 = tp[:].rearrange("p (m i) -> p m i", m=8, i=NH)

                    # Dd mask for tangent diag rows (sample-major)
                    ddt = stg.tile([128, 7], F32, tag="ddt", name="ddt")
                    nc.vector.tensor_single_scalar(ddt[:], tp[:, 0:7], 0.0,
                                                   OP.is_gt)
                    nc.vector.tensor_tensor(
                        tp3[:, 1:8, 0:7], tp3[:, 1:8, 0:7],
                        ddt[:].unsqueeze(1).broadcast_to((128, 7, 7)), OP.mult)

                    # out_L_diag / out_L_tril
                    s_diag = stg.tile([128, 7], F32, tag="s_diag", name="s_diag")
                    nc.scalar.copy(s_diag[:], tp[:, 0:7])
                    nc.sync.dma_start(o_diag[sp:sp + 128, :], s_diag[:])
                    s_tril = stg.tile([128, 21], F32, tag="s_tril", name="s_tril")
                    nc.scalar.copy(s_tril[:], tp[:, 7:28])
                    nc.sync.dma_start(o_tril[sp:sp + 128, :], s_tril[:])

                    # out_L_diag_dq [i*7+k] and out_L_tril_dq [j*7+k]
                    s_ddq = stg.tile([128, 49], F32, tag="s_ddq", name="s_ddq")
                    nc.vector.tensor_copy(
                        s_ddq[:].rearrange("p (i k) -> p k i", i=7, k=7),
                        tp3[:, 1:8, 0:7])
                    nc.sync.dma_start(o_ddq[sp:sp + 128, :], s_ddq[:])
                    s_tdq = stg.tile([128, 147], F32, tag="s_tdq", name="s_tdq")
                    nc.vector.tensor_copy(
                        s_tdq[:].rearrange("p (j k) -> p k j", j=21, k=7),
                        tp3[:, 1:8, 7:28])
                    nc.sync.dma_start(o_tdq[sp:sp + 128, :], s_tdq[:])

                    # L record [i*7+j]: diag at 8i (+0.1), tril rows
                    s_L = stg.tile([128, 56], F32, tag="s_L", name="s_L")
                    nc.gpsimd.memset(s_L[:], 0.0)
                    nc.vector.tensor_scalar_add(
                        s_L[:].rearrange("p (i x) -> p i x", i=7, x=8)[:, :, 0:1],
                        tp[:, 0:7].unsqueeze(2), 0.1)
                    for r in range(1, 7):
                        nc.vector.tensor_copy(
                            s_L[:, 7 * r:7 * r + r],
                            tp[:, 7 + TRIL_OFF[r]:7 + TRIL_OFF[r] + r])
                    nc.sync.dma_start(o_L[sp:sp + 128, :], s_L[:, 0:49])

                    # L_transp: free-dim transpose of the 7x7 record
                    s_Lt = stg.tile([128, 49], F32, tag="s_Lt", name="s_Lt")
                    nc.gpsimd.tensor_copy(
                        s_Lt[:].rearrange("p (i j) -> p i j", i=7, j=7),
                        s_L[:, 0:49].rearrange("p (i j) -> p j i", i=7, j=7))
                    nc.sync.dma_start(o_Lt[sp:sp + 128, :], s_Lt[:])

                    # L_dq record [i*49 + j*7 + k]
                    s_ldq = stg.tile([128, 392], F32, tag="s_ldq", name="s_ldq")
                    nc.gpsimd.memset(s_ldq[:], 0.0)
                    nc.gpsimd.tensor_copy(
                        s_ldq[:].rearrange("p (i x) -> p i x", i=7, x=56)[:, :, 0:7],
                        s_ddq[:].rearrange("p (i k) -> p i k", i=7, k=7))
                    for r in range(1, 7):
                        nc.gpsimd.tensor_copy(
                            s_ldq[:, 49 * r:49 * r + 7 * r],
                            s_tdq[:, 7 * TRIL_OFF[r]:7 * TRIL_OFF[r] + 7 * r])
                    nc.sync.dma_start(o_ldq[sp:sp + 128, :], s_ldq[:, 0:343])

                    # L_dq_transpose: swap i,j
                    s_ldqt = stg.tile([128, 343], F32, tag="s_ldqt", name="s_ldqt")
                    nc.gpsimd.tensor_copy(
                        s_ldqt[:].rearrange("p (j i k) -> p j i k", j=7, i=7, k=7),
                        s_ldq[:, 0:343].rearrange("p (i j k) -> p j i k",
                                                  i=7, j=7, k=7))
                    nc.sync.dma_start(o_ldqt[sp:sp + 128, :], s_ldqt[:])

                    # H = L @ L^T per sample
                    l3 = s_L[:, 0:49].rearrange("p (i k) -> p i k", i=7, k=7)
                    s_h3 = stg.tile([128, 343], F32, tag="tmp343", name="s_h3")
                    nc.vector.tensor_tensor(
                        s_h3[:].rearrange("p (i j k) -> p i j k", i=7, j=7, k=7),
                        l3.unsqueeze(2).broadcast_to((128, 7, 7, 7)),
                        l3.unsqueeze(1).broadcast_to((128, 7, 7, 7)), OP.mult)
                    s_H = stg.tile([128, 49], F32, tag="s_H", name="s_H")
                    nc.vector.tensor_reduce(
                        s_H[:].rearrange("p (i j) -> p i j", i=7, j=7),
                        s_h3[:].rearrange("p (i j k) -> p i j k", i=7, j=7, k=7),
                        mybir.AxisListType.X, OP.add)
                    nc.sync.dma_start(o_H[sp:sp + 128, :], s_H[:])

                    # L_dt = sum_k L_dq[..., k] * qd[k]
                    qdt = stg.tile([128, 7], F32, tag="qdt", name="qdt")
                    nc.sync.dma_start(qdt[:], qd_d[sp:sp + 128, :])
                    s_l3 = stg.tile([128, 343], F32, tag="tmp343", name="s_l3")
                    nc.vector.tensor_tensor(
                        s_l3[:].rearrange("p (a k) -> p a k", a=49, k=7),
                        s_ldq[:, 0:343].rearrange("p (a k) -> p a k", a=49, k=7),
                        qdt[:].unsqueeze(1).broadcast_to((128, 49, 7)), OP.mult)
                    s_ldt = stg.tile([128, 49], F32, tag="s_ldt", name="s_ldt")
                    nc.vector.tensor_reduce(
                        s_ldt[:],
                        s_l3[:].rearrange("p (a k) -> p a k", a=49, k=7),
                        mybir.AxisListType.X, OP.add)
                    nc.sync.dma_start(o_ldt[sp:sp + 128, :], s_ldt[:])

                    # L_dt_transpose
                    s_ldtt = stg.tile([128, 49], F32, tag="s_ldtt", name="s_ldtt")
                    nc.gpsimd.tensor_copy(
                        s_ldtt[:].rearrange("p (i j) -> p i j", i=7, j=7),
                        s_ldt[:].rearrange("p (i j) -> p j i", i=7, j=7))
                    nc.sync.dma_start(o_ldtt[sp:sp + 128, :], s_ldtt[:])

    nc.compile()
    return nc


def _get_nc(repeat=1):
    key = f"nc{repeat}"
    if key not in _NC_CACHE:
        _NC_CACHE[key] = _build_nc(repeat)
    return _NC_CACHE[key]


def _prep_in_maps(q, qd, W0, b0, W1, b1, W2, b2, Wd, bd, Wt, bt):
    q = np.ascontiguousarray(np.asarray(q, np.float32)).reshape(B, N_DOF)
    qd = np.ascontiguousarray(np.asarray(qd, np.float32)).reshape(B, N_DOF)
    W0 = np.asarray(W0, np.float32)
    W1 = np.asarray(W1, np.float32)
    W2 = np.asarray(W2, np.float32)
    Wh = np.concatenate([np.asarray(Wd, np.float32),
                         np.asarray(Wt, np.float32)], axis=0)
    shared = {
        "w0t": np.ascontiguousarray(W0.T),
        "w0c": np.ascontiguousarray(W0),
        "w1t": np.ascontiguousarray(W1.T),
        "w2t": np.ascontiguousarray(W2.T),
        "wht": np.ascontiguousarray(Wh.T),
        "b0m": np.ascontiguousarray(np.asarray(b0, np.float32).reshape(NMT, 128).T),
        "b1m": np.ascontiguousarray(np.asarray(b1, np.float32).reshape(NMT, 128).T),
        "b2m": np.ascontiguousarray(np.asarray(b2, np.float32).reshape(NMT, 128).T),
        "bh": np.ascontiguousarray(
            np.concatenate([np.asarray(bd, np.float32),
                            np.asarray(bt, np.float32)]).reshape(NH, 1)),
    }
    in_maps = []
    for c in range(NCORES):
        sl = slice(c * BC, (c + 1) * BC)
        in_maps.append({"q": q[sl], "qd": qd[sl], **shared})
    return in_maps


def run_cores(in_maps):
    nc = _get_nc()
    return run_bass_kernel_spmd(nc, in_maps, core_ids=list(range(NCORES)))


def kernel(q, qd, qdd, W0, b0, W1, b1, W2, b2, Wd, bd, Wt, bt):
    in_maps = _prep_in_maps(q, qd, W0, b0, W1, b1, W2, b2, Wd, bd, Wt, bt)
    res = run_cores(in_maps).results

    def cat(name):
        return np.concatenate([res[c][name] for c in range(NCORES)], axis=0)

    out_L_diag = cat("o_diag")                        # (B, 7)
    out_L_tril = cat("o_tril")                        # (B, 21)
    out_L_diag_dq = cat("o_ddq").reshape(B, 7, 7)
    out_L_tril_dq = cat("o_tdq").reshape(B, 21, 7)
    L = cat("o_L").reshape(B, 7, 7)
    L_dq = cat("o_ldq").reshape(B, 7, 7, 7)
    L_transp = cat("o_Lt").reshape(B, 7, 7)
    L_dq_transpose = cat("o_ldqt").reshape(B, 7, 7, 7)
    H = cat("o_H").reshape(B, 7, 7)
    L_dt = cat("o_ldt").reshape(B, 7, 7)
    L_dt_transpose = cat("o_ldtt").reshape(B, 7, 7)
    return (out_L_diag, out_L_tril, out_L_diag_dq, out_L_tril_dq,
            L, L_dq, L_transp, L_dq_transpose, H, L_dt, L_dt_transpose)


# revision 6
# speedup vs baseline: 3111.9163x; 48.4006x over previous
"""Deep Lagrangian Network kernel for 8 Trainium2 NeuronCores.

Data-parallel over the batch (32768 samples -> 4096/core). Per core:
  - 3-layer ReLU MLP forward (fp32r matmuls, feature-major layout)
  - forward-mode Jacobian: 7 tangent columns propagated through the trunk
    (seed_k = relu'(z1) * W0[:,k], masked by relu' at each layer)
  - heads (diag 7 + tril 21) for forward and all tangents
  - PE transposes to sample-major, then scatter-assembly of the 11 outputs
    (L, L_dq, transposes, H = L L^T, L_dt = L_dq . qd) as contiguous
    per-sample records, DMA'd out as dense blocks.

kernel(**inputs) takes the FULL unsharded inputs and returns the full
11-tuple matching the reference.
"""
import numpy as np

import concourse.bacc as bacc
import concourse.tile as tile
import concourse.mybir as mybir
from concourse.bass_utils import run_bass_kernel_spmd
from concourse import masks

F32 = mybir.dt.float32
F32R = mybir.dt.float32r
AF = mybir.ActivationFunctionType
OP = mybir.AluOpType

N_DOF = 7
N_TRIL = 21
NH = 28          # head rows: 7 diag + 21 tril
HID = 512
B = 32768
NCORES = 8
BC = B // NCORES          # 4096 samples per core
NS = 512                  # samples per chunk
NCH = BC // NS            # 8 chunks
NMT = HID // 128          # 4 partition tiles per 512 features
# tril row offsets: row r (1..6) covers tril indices off_r .. off_r+r-1
TRIL_OFF = [r * (r - 1) // 2 for r in range(8)]

_NC_CACHE = {}


def _build_nc(repeat=1, mode="full"):
    nc = bacc.Bacc(None, target_bir_lowering=False)

    # ---- per-core DRAM inputs (host pre-transposed layouts)
    q_d = nc.dram_tensor("q", [BC, N_DOF], F32, kind="ExternalInput")
    qd_d = nc.dram_tensor("qd", [BC, N_DOF], F32, kind="ExternalInput")
    w0t_d = nc.dram_tensor("w0t", [N_DOF, HID], F32, kind="ExternalInput")
    w0c_d = nc.dram_tensor("w0c", [HID, N_DOF], F32, kind="ExternalInput")
    w1t_d = nc.dram_tensor("w1t", [HID, HID], F32, kind="ExternalInput")
    w2t_d = nc.dram_tensor("w2t", [HID, HID], F32, kind="ExternalInput")
    wht_d = nc.dram_tensor("wht", [HID, NH], F32, kind="ExternalInput")
    b0_d = nc.dram_tensor("b0m", [128, NMT], F32, kind="ExternalInput")
    b1_d = nc.dram_tensor("b1m", [128, NMT], F32, kind="ExternalInput")
    b2_d = nc.dram_tensor("b2m", [128, NMT], F32, kind="ExternalInput")
    bh_d = nc.dram_tensor("bh", [NH, 1], F32, kind="ExternalInput")

    # ---- per-core DRAM outputs
    o_diag = nc.dram_tensor("o_diag", [BC, 7], F32, kind="ExternalOutput")
    o_tril = nc.dram_tensor("o_tril", [BC, 21], F32, kind="ExternalOutput")
    o_ddq = nc.dram_tensor("o_ddq", [BC, 49], F32, kind="ExternalOutput")
    o_tdq = nc.dram_tensor("o_tdq", [BC, 147], F32, kind="ExternalOutput")
    o_L = nc.dram_tensor("o_L", [BC, 49], F32, kind="ExternalOutput")
    o_Lt = nc.dram_tensor("o_Lt", [BC, 49], F32, kind="ExternalOutput")
    o_ldq = nc.dram_tensor("o_ldq", [BC, 343], F32, kind="ExternalOutput")
    o_ldqt = nc.dram_tensor("o_ldqt", [BC, 343], F32, kind="ExternalOutput")
    o_H = nc.dram_tensor("o_H", [BC, 49], F32, kind="ExternalOutput")
    o_ldt = nc.dram_tensor("o_ldt", [BC, 49], F32, kind="ExternalOutput")
    o_ldtt = nc.dram_tensor("o_ldtt", [BC, 49], F32, kind="ExternalOutput")

    with tile.TileContext(nc) as tc:
        with (
            tc.tile_pool(name="wp", bufs=1) as wp,
            tc.tile_pool(name="act", bufs=2) as act,
            tc.tile_pool(name="msk", bufs=1) as msk,
            tc.tile_pool(name="tan", bufs=2) as tan,
            tc.tile_pool(name="hd", bufs=2) as hd,
            tc.tile_pool(name="stg", bufs=2) as stg,
            tc.tile_pool(name="pmm", bufs=4, space="PSUM") as pmm,
            tc.tile_pool(name="php", bufs=2, space="PSUM") as php,
            tc.tile_pool(name="ptr", bufs=2, space="PSUM") as ptr,
        ):
            # ================= constants / weights =================
            w1f = [wp.tile([128, HID], F32, tag=f"w1f{k}", name=f"w1f{k}")
                   for k in range(NMT)]
            w2f = [wp.tile([128, HID], F32, tag=f"w2f{k}", name=f"w2f{k}")
                   for k in range(NMT)]
            whf = [wp.tile([128, NH], F32, tag=f"whf{k}", name=f"whf{k}")
                   for k in range(NMT)]
            w1r = [wp.tile([128, HID], F32R, tag=f"w1r{k}", name=f"w1r{k}")
                   for k in range(NMT)]
            w2r = [wp.tile([128, HID], F32R, tag=f"w2r{k}", name=f"w2r{k}")
                   for k in range(NMT)]
            whr = [wp.tile([128, NH], F32R, tag=f"whr{k}", name=f"whr{k}")
                   for k in range(NMT)]
            w0c = [wp.tile([128, N_DOF], F32, tag=f"w0c{k}", name=f"w0c{k}")
                   for k in range(NMT)]
            for k in range(NMT):
                sl = slice(k * 128, (k + 1) * 128)
                nc.sync.dma_start(w1f[k][:], w1t_d[sl, :])
                nc.vector.tensor_copy(w1r[k][:], w1f[k][:])
                nc.sync.dma_start(w2f[k][:], w2t_d[sl, :])
                nc.vector.tensor_copy(w2r[k][:], w2f[k][:])
                nc.sync.dma_start(whf[k][:], wht_d[sl, :])
                nc.vector.tensor_copy(whr[k][:], whf[k][:])
                nc.sync.dma_start(w0c[k][:], w0c_d[sl, :])
            w0f = wp.tile([N_DOF, HID], F32, tag="w0f")
            nc.sync.dma_start(w0f[:], w0t_d[:])
            b0t = wp.tile([128, NMT], F32, tag="b0t")
            nc.sync.dma_start(b0t[:], b0_d[:])
            b1t = wp.tile([128, NMT], F32, tag="b1t")
            nc.sync.dma_start(b1t[:], b1_d[:])
            b2t = wp.tile([128, NMT], F32, tag="b2t")
            nc.sync.dma_start(b2t[:], b2_d[:])
            bht = wp.tile([NH, 1], F32, tag="bht")
            nc.sync.dma_start(bht[:], bh_d[:])
            ident = wp.tile([NH, NH], F32, tag="ident")
            masks.make_identity(nc, ident[:])

            # ================= main loop over chunks =================
            for c in [cc % NCH for cc in range(NCH * repeat)]:
                s0 = c * NS
                # --- load q chunk transposed [7, NS], round to f32r
                qt = act.tile([N_DOF, NS], F32, tag="qt", name="qt")
                nc.sync.dma_start(qt[:], q_d[s0:s0 + NS, :].rearrange("s d -> d s"))

                # --- layer 1 forward: h1 = relu(W0 @ qT + b0)
                h1 = []
                for mt in range(NMT):
                    zp = pmm.tile([128, NS], F32, tag="mm", name="zp")
                    nc.tensor.matmul(zp[:], w0f[:, mt * 128:(mt + 1) * 128],
                                     qt[:], start=True, stop=True)
                    h = act.tile([128, NS], F32, tag=f"h1_{mt}", name=f"h1_{mt}")
                    nc.scalar.activation(h[:], zp[:], AF.Relu,
                                         bias=b0t[:, mt:mt + 1], scale=1.0)
                    h1.append(h)

                # --- layer 2 forward
                h2, d2 = [], []
                for mt in range(NMT):
                    zp = pmm.tile([128, NS], F32, tag="mm", name="zp")
                    for kc in range(NMT):
                        nc.tensor.matmul(zp[:],
                                         w1f[kc][:, mt * 128:(mt + 1) * 128],
                                         h1[kc][:],
                                         start=(kc == 0), stop=(kc == NMT - 1))
                    h = act.tile([128, NS], F32, tag=f"h2_{mt}", name=f"h2_{mt}")
                    nc.scalar.activation(h[:], zp[:], AF.Relu,
                                         bias=b1t[:, mt:mt + 1], scale=1.0)
                    h2.append(h)
                    d = msk.tile([128, NS], F32, tag=f"d2_{mt}", name=f"d2_{mt}")
                    nc.gpsimd.tensor_single_scalar(d[:], h[:], 0.0,
                                                   OP.is_gt)
                    d2.append(d)

                # --- layer 3 forward
                h3, d3 = [], []
                for mt in range(NMT):
                    zp = pmm.tile([128, NS], F32, tag="mm", name="zp")
                    for kc in range(NMT):
                        nc.tensor.matmul(zp[:],
                                         w2f[kc][:, mt * 128:(mt + 1) * 128],
                                         h2[kc][:],
                                         start=(kc == 0), stop=(kc == NMT - 1))
                    h = act.tile([128, NS], F32, tag=f"h3_{mt}", name=f"h3_{mt}")
                    nc.scalar.activation(h[:], zp[:], AF.Relu,
                                         bias=b2t[:, mt:mt + 1], scale=1.0)
                    h3.append(h)
                    d = msk.tile([128, NS], F32, tag=f"d3_{mt}", name=f"d3_{mt}")
                    nc.gpsimd.tensor_single_scalar(d[:], h[:], 0.0,
                                                   OP.is_gt)
                    d3.append(d)

                # --- forward head: [diag relu'd; tril] in fm layout [28, NS]
                hp = php.tile([NH, NS], F32, tag="hp", name="hp")
                for kc in range(NMT):
                    nc.tensor.matmul(hp[:], whf[kc][:], h3[kc][:],
                                     start=(kc == 0), stop=(kc == NMT - 1))
                hf = hd.tile([NH, NS], F32, tag="hf", name="hf")
                nc.scalar.activation(hf[:], hp[:], AF.Identity,
                                     bias=bht[:], scale=1.0)
                nc.scalar.activation(hf[0:7, :], hp[0:7, :], AF.Relu,
                                     bias=bht[0:7, :], scale=1.0)

                # --- tangent propagation, one tangent direction k at a time
                hts = []
                for k in range(N_DOF if mode in ("full", "notasm") else 0):
                    seeds = []
                    for mt in range(NMT):
                        sd = tan.tile([128, NS], F32R, tag=f"sd_{mt}",
                                      name=f"sd_{mt}")
                        nc.gpsimd.tensor_scalar(sd[:], h1[mt][:],
                                                0.0, w0c[mt][:, k:k + 1],
                                                OP.is_gt, OP.mult)
                        seeds.append(sd)
                    a2 = []
                    for mt in range(NMT):
                        zp = pmm.tile([128, NS], F32, tag="mm", name="zp")
                        for kc in range(NMT):
                            nc.tensor.matmul(zp[:],
                                             w1r[kc][:, mt * 128:(mt + 1) * 128],
                                             seeds[kc][:],
                                             start=(kc == 0), stop=(kc == NMT - 1))
                        a = tan.tile([128, NS], F32R, tag=f"a2_{mt}",
                                     name=f"a2_{mt}")
                        nc.vector.tensor_tensor(a[:], zp[:], d2[mt][:], OP.mult)
                        a2.append(a)
                    a3 = []
                    for mt in range(NMT):
                        zp = pmm.tile([128, NS], F32, tag="mm", name="zp")
                        for kc in range(NMT):
                            nc.tensor.matmul(zp[:],
                                             w2r[kc][:, mt * 128:(mt + 1) * 128],
                                             a2[kc][:],
                                             start=(kc == 0), stop=(kc == NMT - 1))
                        a = tan.tile([128, NS], F32R, tag=f"a3_{mt}",
                                     name=f"a3_{mt}")
                        nc.vector.tensor_tensor(a[:], zp[:], d3[mt][:], OP.mult)
                        a3.append(a)
                    hp_k = php.tile([NH, NS], F32, tag="hp", name="hp_k")
                    for kc in range(NMT):
                        nc.tensor.matmul(hp_k[:], whr[kc][:], a3[kc][:],
                                         start=(kc == 0), stop=(kc == NMT - 1))
                    ht = hd.tile([NH, NS], F32, tag=f"ht_{k}", name=f"ht_{k}")
                    nc.scalar.copy(ht[:], hp_k[:])
                    hts.append(ht)

                # --- per 128-sample tile: transpose + assemble + DMA out
                if mode in ("notasm", "fwdonly"):
                    # minimal output so nothing is dead: dump hf and tangent heads
                    st_min = stg.tile([NH, NS], F32, tag="st_min", name="st_min")
                    nc.vector.tensor_copy(st_min[:], hf[:])
                    for ht_ in hts:
                        nc.vector.tensor_tensor(st_min[:], st_min[:], ht_[:], OP.add)
                    nc.sync.dma_start(
                        o_ddq[s0:s0 + NS, 0:NH].rearrange("s d -> d s"), st_min[:])
                    continue
                for p in range(NS // 128):
                    sp = s0 + p * 128
                    csl = slice(p * 128, (p + 1) * 128)
                    tp = ptr.tile([128, 8 * NH], F32, tag="tp", name="tp")
                    nc.tensor.transpose(tp[:, 0:NH], hf[:, csl], ident[:])
                    for k in range(N_DOF):
                        nc.tensor.transpose(tp[:, (k + 1) * NH:(k + 2) * NH],
                                            hts[k][:, csl], ident[:])
                    tp3 = tp[:].rearrange("p (m i) -> p m i", m=8, i=NH)

                    # Dd mask for tangent diag rows (sample-major)
                    ddt = stg.tile([128, 7], F32, tag="ddt", name="ddt")
                    nc.vector.tensor_single_scalar(ddt[:], tp[:, 0:7], 0.0,
                                                   OP.is_gt)
                    nc.vector.tensor_tensor(
                        tp3[:, 1:8, 0:7], tp3[:, 1:8, 0:7],
                        ddt[:].unsqueeze(1).broadcast_to((128, 7, 7)), OP.mult)

                    # out_L_diag / out_L_tril
                    s_diag = stg.tile([128, 7], F32, tag="s_diag", name="s_diag")
                    nc.scalar.copy(s_diag[:], tp[:, 0:7])
                    nc.sync.dma_start(o_diag[sp:sp + 128, :], s_diag[:])
                    s_tril = stg.tile([128, 21], F32, tag="s_tril", name="s_tril")
                    nc.scalar.copy(s_tril[:], tp[:, 7:28])
                    nc.sync.dma_start(o_tril[sp:sp + 128, :], s_tril[:])

                    # out_L_diag_dq [i*7+k] and out_L_tril_dq [j*7+k]
                    s_ddq = stg.tile([128, 49], F32, tag="s_ddq", name="s_ddq")
                    nc.vector.tensor_copy(
                        s_ddq[:].rearrange("p (i k) -> p k i", i=7, k=7),
                        tp3[:, 1:8, 0:7])
                    nc.sync.dma_start(o_ddq[sp:sp + 128, :], s_ddq[:])
                    s_tdq = stg.tile([128, 147], F32, tag="s_tdq", name="s_tdq")
                    nc.vector.tensor_copy(
                        s_tdq[:].rearrange("p (j k) -> p k j", j=21, k=7),
                        tp3[:, 1:8, 7:28])
                    nc.sync.dma_start(o_tdq[sp:sp + 128, :], s_tdq[:])

                    # L record [i*7+j]: diag at 8i (+0.1), tril rows
                    s_L = stg.tile([128, 56], F32, tag="s_L", name="s_L")
                    nc.gpsimd.memset(s_L[:], 0.0)
                    nc.vector.tensor_scalar_add(
                        s_L[:].rearrange("p (i x) -> p i x", i=7, x=8)[:, :, 0:1],
                        tp[:, 0:7].unsqueeze(2), 0.1)
                    for r in range(1, 7):
                        nc.vector.tensor_copy(
                            s_L[:, 7 * r:7 * r + r],
                            tp[:, 7 + TRIL_OFF[r]:7 + TRIL_OFF[r] + r])
                    nc.sync.dma_start(o_L[sp:sp + 128, :], s_L[:, 0:49])

                    # L_transp: free-dim transpose of the 7x7 record
                    s_Lt = stg.tile([128, 49], F32, tag="s_Lt", name="s_Lt")
                    nc.gpsimd.tensor_copy(
                        s_Lt[:].rearrange("p (i j) -> p i j", i=7, j=7),
                        s_L[:, 0:49].rearrange("p (i j) -> p j i", i=7, j=7))
                    nc.sync.dma_start(o_Lt[sp:sp + 128, :], s_Lt[:])

                    # L_dq record [i*49 + j*7 + k]
                    s_ldq = stg.tile([128, 392], F32, tag="s_ldq", name="s_ldq")
                    nc.gpsimd.memset(s_ldq[:], 0.0)
                    nc.gpsimd.tensor_copy(
                        s_ldq[:].rearrange("p (i x) -> p i x", i=7, x=56)[:, :, 0:7],
                        s_ddq[:].rearrange("p (i k) -> p i k", i=7, k=7))
                    for r in range(1, 7):
                        nc.gpsimd.tensor_copy(
                            s_ldq[:, 49 * r:49 * r + 7 * r],
                            s_tdq[:, 7 * TRIL_OFF[r]:7 * TRIL_OFF[r] + 7 * r])
                    nc.sync.dma_start(o_ldq[sp:sp + 128, :], s_ldq[:, 0:343])

                    # L_dq_transpose: swap i,j
                    s_ldqt = stg.tile([128, 343], F32, tag="s_ldqt", name="s_ldqt")
                    nc.gpsimd.tensor_copy(
                        s_ldqt[:].rearrange("p (j i k) -> p j i k", j=7, i=7, k=7),
                        s_ldq[:, 0:343].rearrange("p (i j k) -> p j i k",
                                                  i=7, j=7, k=7))
                    nc.sync.dma_start(o_ldqt[sp:sp + 128, :], s_ldqt[:])

                    # H = L @ L^T per sample
                    l3 = s_L[:, 0:49].rearrange("p (i k) -> p i k", i=7, k=7)
                    s_h3 = stg.tile([128, 343], F32, tag="tmp343", name="s_h3")
                    nc.vector.tensor_tensor(
                        s_h3[:].rearrange("p (i j k) -> p i j k", i=7, j=7, k=7),
                        l3.unsqueeze(2).broadcast_to((128, 7, 7, 7)),
                        l3.unsqueeze(1).broadcast_to((128, 7, 7, 7)), OP.mult)
                    s_H = stg.tile([128, 49], F32, tag="s_H", name="s_H")
                    nc.vector.tensor_reduce(
                        s_H[:].rearrange("p (i j) -> p i j", i=7, j=7),
                        s_h3[:].rearrange("p (i j k) -> p i j k", i=7, j=7, k=7),
                        mybir.AxisListType.X, OP.add)
                    nc.sync.dma_start(o_H[sp:sp + 128, :], s_H[:])

                    # L_dt = sum_k L_dq[..., k] * qd[k]
                    qdt = stg.tile([128, 7], F32, tag="qdt", name="qdt")
                    nc.sync.dma_start(qdt[:], qd_d[sp:sp + 128, :])
                    s_l3 = stg.tile([128, 343], F32, tag="tmp343", name="s_l3")
                    nc.vector.tensor_tensor(
                        s_l3[:].rearrange("p (a k) -> p a k", a=49, k=7),
                        s_ldq[:, 0:343].rearrange("p (a k) -> p a k", a=49, k=7),
                        qdt[:].unsqueeze(1).broadcast_to((128, 49, 7)), OP.mult)
                    s_ldt = stg.tile([128, 49], F32, tag="s_ldt", name="s_ldt")
                    nc.vector.tensor_reduce(
                        s_ldt[:],
                        s_l3[:].rearrange("p (a k) -> p a k", a=49, k=7),
                        mybir.AxisListType.X, OP.add)
                    nc.sync.dma_start(o_ldt[sp:sp + 128, :], s_ldt[:])

                    # L_dt_transpose
                    s_ldtt = stg.tile([128, 49], F32, tag="s_ldtt", name="s_ldtt")
                    nc.gpsimd.tensor_copy(
                        s_ldtt[:].rearrange("p (i j) -> p i j", i=7, j=7),
                        s_ldt[:].rearrange("p (i j) -> p j i", i=7, j=7))
                    nc.sync.dma_start(o_ldtt[sp:sp + 128, :], s_ldtt[:])

    nc.compile()
    return nc


def _get_nc(repeat=1, mode="full"):
    key = f"nc{repeat}_{mode}"
    if key not in _NC_CACHE:
        _NC_CACHE[key] = _build_nc(repeat, mode)
    return _NC_CACHE[key]


def _prep_in_maps(q, qd, W0, b0, W1, b1, W2, b2, Wd, bd, Wt, bt):
    q = np.ascontiguousarray(np.asarray(q, np.float32)).reshape(B, N_DOF)
    qd = np.ascontiguousarray(np.asarray(qd, np.float32)).reshape(B, N_DOF)
    W0 = np.asarray(W0, np.float32)
    W1 = np.asarray(W1, np.float32)
    W2 = np.asarray(W2, np.float32)
    Wh = np.concatenate([np.asarray(Wd, np.float32),
                         np.asarray(Wt, np.float32)], axis=0)
    shared = {
        "w0t": np.ascontiguousarray(W0.T),
        "w0c": np.ascontiguousarray(W0),
        "w1t": np.ascontiguousarray(W1.T),
        "w2t": np.ascontiguousarray(W2.T),
        "wht": np.ascontiguousarray(Wh.T),
        "b0m": np.ascontiguousarray(np.asarray(b0, np.float32).reshape(NMT, 128).T),
        "b1m": np.ascontiguousarray(np.asarray(b1, np.float32).reshape(NMT, 128).T),
        "b2m": np.ascontiguousarray(np.asarray(b2, np.float32).reshape(NMT, 128).T),
        "bh": np.ascontiguousarray(
            np.concatenate([np.asarray(bd, np.float32),
                            np.asarray(bt, np.float32)]).reshape(NH, 1)),
    }
    in_maps = []
    for c in range(NCORES):
        sl = slice(c * BC, (c + 1) * BC)
        in_maps.append({"q": q[sl], "qd": qd[sl], **shared})
    return in_maps


def run_cores(in_maps):
    nc = _get_nc()
    return run_bass_kernel_spmd(nc, in_maps, core_ids=list(range(NCORES)))


def kernel(q, qd, qdd, W0, b0, W1, b1, W2, b2, Wd, bd, Wt, bt):
    in_maps = _prep_in_maps(q, qd, W0, b0, W1, b1, W2, b2, Wd, bd, Wt, bt)
    res = run_cores(in_maps).results

    def cat(name):
        return np.concatenate([res[c][name] for c in range(NCORES)], axis=0)

    out_L_diag = cat("o_diag")                        # (B, 7)
    out_L_tril = cat("o_tril")                        # (B, 21)
    out_L_diag_dq = cat("o_ddq").reshape(B, 7, 7)
    out_L_tril_dq = cat("o_tdq").reshape(B, 21, 7)
    L = cat("o_L").reshape(B, 7, 7)
    L_dq = cat("o_ldq").reshape(B, 7, 7, 7)
    L_transp = cat("o_Lt").reshape(B, 7, 7)
    L_dq_transpose = cat("o_ldqt").reshape(B, 7, 7, 7)
    H = cat("o_H").reshape(B, 7, 7)
    L_dt = cat("o_ldt").reshape(B, 7, 7)
    L_dt_transpose = cat("o_ldtt").reshape(B, 7, 7)
    return (out_L_diag, out_L_tril, out_L_diag_dq, out_L_tril_dq,
            L, L_dq, L_transp, L_dq_transpose, H, L_dt, L_dt_transpose)
